# revision 36
# baseline (speedup 1.0000x reference)
"""Trainium2 Bass kernel for nn_MessageFunction (gnn_message_passing).

Math (validated against the reference):
  The reference broadcasts h_w[:, :, None] -> (B*N, IN_F, N) and reshapes to
  [E, IN_F]; row-major order makes every row constant:
      h_w_rows[e, i] = h_w.reshape(-1)[e]   for all i.
  Hence the per-edge bmm collapses:
      m[e, o] = sum_i edge_output[e, o, i] * s[e]
              = s[e] * (x3[e] @ W4s[:, o] + b4s[o])
  with W4s = W4.reshape(HID3, OUT_F, IN_F).sum(-1), b4s = b4.reshape(OUT_F,
  IN_F).sum(-1), s = h_w.reshape(-1).  This is an exact reassociation (only
  rounding differences) and removes the [E,128]@[128,4096] matmul + bmm.

Kernel: data-parallel over E = 32768 edges, 4096 per core across 8 cores,
MLP weights replicated, no cross-core communication.  Per core the MLP runs
features-on-partitions with edges streaming on the free dim:
    x1 = relu(W1.T @ eT)        K=32  bf16 -> [128, e]
    x2 = relu(W2.T @ x1)        K=128 f32r -> [256, e] (merged 2-bank PSUM)
    x3 = relu(W3.T @ x2)        K=256 f32r -> [128, e] (PSUM accumulation)
    y  = blockdiag(W4s,W4s).T @ [x3(2p); x3(2p+1)]  -> [128, e] per pair
    out = y * s                 tensor_mul on VectorE
L1 runs in bf16 (e_vw is the big input stream; bf16 halves it, and L1's
PSUM accumulation stays f32).  The rest runs float32r: bf16-out pointwise
passes measured ~20% slower on both ScalarE and DVE, and pointwise is the
steady-state co-bottleneck (the PSUM read port pins DVE at 1x), so keeping
f32 outputs is the faster choice.  The per-edge scale s_b is uploaded
pre-broadcast (1 MB) but ordered LAST on the scalar ring, so it streams
during early compute instead of gating the weight/input loads (a stride-0
on-chip broadcast DMA was tried and abandoned: HWDGE distributes work over
the 16 DMA engines by the outermost AP dim, and a [2, 64, N] broadcast
source lands on 2 engines at ~20 GB/s each).  The emission is
software-pipelined with a one-iteration skew per stage; relu passes are
split between ScalarE and VectorE; a few dependency-free warm-up matmuls
cover the input-load window so the PE HAM clock gate is released before
the real stream starts.
"""

import os

import ml_dtypes
import numpy as np

import concourse.bacc as bacc
import concourse.bass as bass
import concourse.mybir as mybir
import concourse.tile as tile
from concourse.bass_utils import run_bass_kernel_spmd
from concourse.vector_clock import ScopedClock


def _ensure_ntff_hook_module():
    """run_bass_kernel_spmd(trace=True) (or BASS_TRACE=1 in the environment)
    imports antenv.axon_hooks, which is absent from this container's antenv.
    Provide a best-effort stand-in so tracing degrades gracefully (or works,
    when the axon .so exposes the NRT profile symbols)."""
    import sys
    import types

    try:
        import antenv.axon_hooks  # noqa: F401
        return
    except ImportError:
        pass
    try:
        import antenv
    except ImportError:
        return
    hook = None
    try:
        from trn_agent_boot.trn_boot import _ntff_profile_via_ctypes

        hook = _ntff_profile_via_ctypes("/opt/axon/libaxon_pjrt.so")
    except Exception:
        hook = None
    mod = types.ModuleType("antenv.axon_hooks")
    state = {"hook": hook}
    mod.set_axon_ntff_profile_hook = lambda h: state.__setitem__("hook", h)
    mod.get_axon_ntff_profile_hook = lambda: state["hook"]
    sys.modules["antenv.axon_hooks"] = mod
    antenv.axon_hooks = mod


_ensure_ntff_hook_module()


def _guard_upload_artifacts():
    """The trace path uploads the NEFF dir to a cloud bucket, which this
    container cannot reach; fall back to the local path instead of raising."""
    import concourse.bass_utils as bu

    orig = bu.upload_artifacts

    def safe_upload(tmpdir):
        try:
            return orig(tmpdir)
        except Exception:
            return tmpdir

    bu.upload_artifacts = safe_upload


_guard_upload_artifacts()


def _patch_walrus_max_sem():
    """Cap the semaphore file size walrus codegen manages.  Its NEFF exit
    epilogue clears every semaphore up to the cap, one EVENT_SEMAPHORE per
    sem split across the 5 engines (~115 ns each on TensorE) — with the
    default cap of 256 that is ~5 us of pure exit overhead.  This kernel
    only uses sems 150-166."""
    import concourse.bass_utils as bu

    orig = bu.run_command

    def patched(argv, **kw):
        try:
            if (
                argv
                and "walrus_driver" in str(argv[0])
                and any("codegen" in str(a) for a in argv)
                and not any(str(a).startswith("--max-sem-num") for a in argv)
            ):
                argv = list(argv) + ["--max-sem-num=176"]
        except Exception:
            pass
        return orig(argv, **kw)

    bu.run_command = patched


_patch_walrus_max_sem()


def _fast_drain_and_barrier(self, tick_clock, wait_clock):
    """Tile exit with sem-only barriers.

    The stock exit emits two full all-engine barriers (per-engine InstDrain +
    EVSEM butterfly, ~2-3us each).  Each engine's pipeline is already flushed
    by its own per-op DRAINs, and the Sync drain below waits on every
    semaphore (including DMA completions), so sequencer-level barriers are
    sufficient here and several microseconds cheaper.
    """
    drain_inst = self.nc.sync.drain()
    wait_clock.add_sem_waits(
        drain_inst.ins, ScopedClock({None: tick_clock.global_clock})
    )
    self.nc.all_engine_barrier(sem_only=True)
    popped = self.nc._tile_sem_poison_stack.pop()
    assert popped is self._sem_poison
    # No tile-sem clear here: NRT's injected NEFF epilogue clears the whole
    # semaphore file [2..255] on every execution anyway (observed ~50 clears
    # per engine after the final barrier), so the range-clear + trailing
    # barrier are redundant.  Free the handles bookkeeping-only.
    sems = list(self.sems.allocated().values())
    sem_nums = [s.num if hasattr(s, "num") else s for s in sems]
    self.nc._state.prepend_free_semaphores(sem_nums)
    for poison_set in self.nc._tile_sem_poison_stack:
        poison_set.update(sem_nums)


tile.TileContext._drain_and_barrier = _fast_drain_and_barrier

# Problem constants (hardcoded per the harness contract).
B, N = 8, 64
IN_F, OUT_F = 64, 64
EDGE_F = 32
HID1, HID2, HID3 = 128, 256, 128
E = B * N * N            # 32768
N_CORES = 8
E_LOC = E // N_CORES     # 4096
TILE = 512               # edges per tile (one PSUM bank per stage)
NT = E_LOC // TILE       # 8 tiles per core

F32 = mybir.dt.float32
# Core matmul dtype: float32r streams at 1 cycle/row for N>=256 (same as
# bf16) with much better precision than bf16.
DT = mybir.dt.float32r
# L1 input dtype: bf16 halves the e_vw stream (the big per-edge input).
DT1 = mybir.dt.bfloat16
NP_BF16 = ml_dtypes.bfloat16

# Packed f32r weight layout: [W2 | W3packed | W4sA | W4sB | b4blk].
WPACK = HID2 + 2 * HID3 + 2 * 128 + 128  # 896

# Module global: last BassKernelResults (test.py reads exec_time_ns from it).
LAST_RESULTS = None


def _build_bass(b2_halves_equal=True, b4_nonzero=False):
    nc = bacc.Bacc(
        "TRN2", target_bir_lowering=False, debug=False, num_devices=N_CORES
    )

    # Per-core inputs.  e_t = e_vw_local.T as [32, 4096] bf16.
    e_t = nc.dram_tensor("e_t", [EDGE_F, E_LOC], DT1, kind="ExternalInput")
    # s pair-stacked: rows 0-63 broadcast s of even tiles, 64-127 odd tiles;
    # column 512*p + c maps to edges 1024p + c (rows<64) / 1024p + 512 + c.
    # bf16: halves the 1 MB upload; the mul output stays f32 (mixed-dtype
    # tensor_tensor casts in1 on read).
    s_b = nc.dram_tensor("s_b", [128, E_LOC // 2], DT1, kind="ExternalInput")
    # s duplicated on two partitions for the optional K=2 bias matmul.
    srd = nc.dram_tensor("srd", [2, E_LOC // 2], DT, kind="ExternalInput")
    w1d = nc.dram_tensor("w1d", [EDGE_F, HID1], DT1, kind="ExternalInput")
    wpd = nc.dram_tensor("wpd", [128, WPACK], DT, kind="ExternalInput")
    # f32 bias columns: b1, b2[:128], b2[128:], b3, b4s.
    auxd = nc.dram_tensor("auxd", [128, 5], F32, kind="ExternalInput")
    outd = nc.dram_tensor(
        "outd", [NT // 2, 128, TILE], F32, kind="ExternalOutput"
    )

    with tile.TileContext(nc) as tc:
        with (
            tc.tile_pool(name="wp", bufs=1) as wp,
            tc.tile_pool(name="acts", bufs=4) as acts,
            tc.tile_pool(name="ps", bufs=1, space="PSUM") as ps,
        ):
            e4 = wp.tile([EDGE_F, E_LOC], DT1, tag="e4")
            w1 = wp.tile([EDGE_F, HID1], DT1, tag="w1")
            wpk = wp.tile([128, WPACK], DT, tag="wpk")
            aux = wp.tile([128, 5], F32, tag="aux")
            sr2 = wp.tile([2, E_LOC // 2], DT, tag="sr2")
            s_sb = wp.tile([128, E_LOC // 2], DT1, tag="s_sb")
            out_sb = wp.tile([128, E_LOC // 2], F32, tag="out_sb")
            # Views into the packed f32r weight tile.
            w2 = wpk[:, 0:HID2]
            w3 = wpk[:, HID2 : HID2 + 2 * HID3]
            w4a = wpk[:, HID2 + 2 * HID3 : HID2 + 2 * HID3 + 128]
            w4b = wpk[:, HID2 + 2 * HID3 + 128 : HID2 + 2 * HID3 + 256]
            b4mm = wpk[0:2, HID2 + 2 * HID3 + 256 : HID2 + 2 * HID3 + 384]
            bb = aux

            # Input loads on the two HWDGE rings (Sync + Scalar); each ring
            # is FIFO end-to-end, so order == need-order.  Sync carries the
            # edge stream (+ output later); Scalar the weights, then s_b —
            # s_b is 1 MB but is not needed until the first L4 (~+4us), so
            # it streams behind the weights, off the critical path.
            nc.scalar.dma_start(w1[:], w1d[:])
            nc.sync.dma_start(e4[:, 0:1024], e_t[:, 0:1024])
            nc.sync.dma_start(e4[:, 1024:4096], e_t[:, 1024:4096])
            nc.scalar.dma_start(aux[:], auxd[:])
            nc.sync.dma_start(wpk[:], wpd[:])
            if b4_nonzero:
                nc.scalar.dma_start(sr2[:], srd[:])
            H = E_LOC // 4  # 1024
            nc.scalar.dma_start(s_sb[:, 0:H], s_b[:, 0:H])
            nc.scalar.dma_start(s_sb[:, H : 2 * H], s_b[:, H : 2 * H])

            # PE warm-up: dependency-free matmuls on scratch data bridge the
            # input-load window so the HAM clock gate releases (first ~3.4us
            # of PE activity runs at 1.2 GHz regardless; burn it on scratch
            # while the DMAs land).  Garbage values are fine — the scratch
            # PSUM is never read.
            scratch = wp.tile([128, TILE], DT, tag="scratch")
            nc.gpsimd.memset(scratch[:].bitcast(F32), 1.0)

            def emit_dummies(n, tag="x3y4"):
                for _ in range(n):
                    warm_ps = ps.tile([128, TILE], F32, tag=tag, bufs=2)
                    nc.tensor.matmul(warm_ps[:], scratch[:, 0:128], scratch[:])

            # The HAM clock gate needs ~3.4us of solid FULL-ARRAY (K=128)
            # matmul duty to release — a gapless K=32 L1 stream measured NO
            # release for 16us.  Ten back-to-back f32r dummies flip the gate
            # deterministically (~3.9us) while the input DMAs land.
            emit_dummies(9)

            def relu_pass(dst, src, bias_col, eng):
                if eng == "A":
                    nc.scalar.activation(
                        dst, src, mybir.ActivationFunctionType.Relu, bias=bias_col
                    )
                else:
                    nc.vector.tensor_scalar(
                        out=dst,
                        in0=src,
                        scalar1=bias_col,
                        scalar2=0.0,
                        op0=mybir.AluOpType.add,
                        op1=mybir.AluOpType.max,
                    )

            x1_t = [None] * NT
            x2_t = [None] * NT
            x3_t = [None] * NT

            # Phase 1: ALL eight L1 tiles as one solid bf16 block.  This is
            # the HAM warm-up (back-to-back matmuls, no dependency gaps)
            # AND it confines the bf16<->f32r LDWEIGHTS mode switch to a
            # single point — interleaving bf16 L1s into the f32r stream
            # measured a ~300ns PE bubble at every switch.  P1 passes
            # alternate ScalarE/VectorE so the x1p banks drain at L1 pace.
            for t in range(NT):
                cs1 = slice(t * TILE, (t + 1) * TILE)
                x1p = ps.tile([128, TILE], F32, tag="x1p", bufs=2)
                nc.tensor.matmul(x1p[:], w1[:], e4[:, cs1])
                x1 = acts.tile([128, TILE], DT, tag="x1", bufs=8)
                relu_pass(x1[:], x1p[:], bb[:, 0:1], "A" if t < 2 else "V")
                x1_t[t] = x1

            # Phase 2: pure-f32r software pipeline over L2/L3/L4.  ScalarE
            # carries the merged P2 pass, VectorE carries P3 + the final
            # muls — both land at ~1.1us/tile vs the PE's ~1.16us/tile, so
            # this phase is PE-bound.  Bridge dummies pad the fill.
            bridge = {0: 2, 1: 2, 2: 1, 3: 1, 5: 1}

            for i in range(NT + 5):
                if i in bridge:
                    emit_dummies(bridge[i])

                # S2 + P2 for tile i (merged 2-bank PSUM, single pass).
                j = i
                if 0 <= j < NT:
                    x2p = ps.tile([128, 2 * TILE], F32, tag="x2p", bufs=2)
                    nc.tensor.matmul(x2p[:, 0:TILE], w2[:, 0:128], x1_t[j][:])
                    nc.tensor.matmul(x2p[:, TILE : 2 * TILE], w2[:, 128:256], x1_t[j][:])
                    x2 = acts.tile([128, 2 * TILE], DT, tag="x2")
                    # A per-partition bias is constant along the free dim, so
                    # one merged pass is only valid when both b2 halves agree
                    # (always true for the zero biases here); otherwise fall
                    # back to two passes.
                    if b2_halves_equal:
                        nc.scalar.activation(
                            x2[:], x2p[:],
                            mybir.ActivationFunctionType.Relu, bias=bb[:, 1:2],
                        )
                    else:
                        nc.scalar.activation(
                            x2[:, 0:TILE], x2p[:, 0:TILE],
                            mybir.ActivationFunctionType.Relu, bias=bb[:, 1:2],
                        )
                        nc.scalar.activation(
                            x2[:, TILE : 2 * TILE], x2p[:, TILE : 2 * TILE],
                            mybir.ActivationFunctionType.Relu, bias=bb[:, 2:3],
                        )
                    x2_t[j] = x2
                    x1_t[j] = None

                # S3 + P3 for tile i-2 (skew hides ScalarE's P2 latency)
                j = i - 2
                if 0 <= j < NT:
                    x3ps = ps.tile([128, TILE], F32, tag="x3y4", bufs=2)
                    nc.tensor.matmul(
                        x3ps[:], w3[:, 0:128], x2_t[j][:, 0:TILE],
                        start=True, stop=False,
                    )
                    nc.tensor.matmul(
                        x3ps[:], w3[:, 128:256], x2_t[j][:, TILE : 2 * TILE],
                        start=False, stop=True,
                    )
                    x3 = acts.tile([128, TILE], DT, tag="x3")
                    # Last two P3s go to ScalarE, which is idle by then —
                    # VectorE still owes the final muls after the last L4.
                    relu_pass(x3[:], x3ps[:], bb[:, 3:4], "A" if j >= 6 else "V")
                    x3_t[j] = x3
                    x2_t[j] = None

                # S4 + P4 for the tile PAIR ending at tile i-4 (odd tiles).
                # Block-diagonal weights put pair tiles (2p, 2p+1) on PSUM
                # partition halves [0:64) / [64:128) of one bank; one
                # full-width tensor_mul applies the per-edge h_w scale.
                j = i - 4
                if 0 <= j < NT and j % 2 == 1:
                    p = j // 2
                    cs = slice(p * TILE, (p + 1) * TILE)
                    y4p = ps.tile([128, TILE], F32, tag="x3y4", bufs=2)
                    nc.tensor.matmul(
                        y4p[:], w4a[:], x3_t[j - 1][:], start=True, stop=False
                    )
                    nc.tensor.matmul(
                        y4p[:], w4b[:], x3_t[j][:],
                        start=False, stop=not b4_nonzero,
                    )
                    if b4_nonzero:
                        # += b4s[o] * s[e] per block, via a K=2 matmul:
                        # lhsT rows = [b4s|0], [0|b4s]; rhs rows = s even/odd.
                        nc.tensor.matmul(
                            y4p[:], b4mm, sr2[:, cs], start=False, stop=True
                        )
                    nc.vector.tensor_mul(out_sb[:, cs], y4p[:], s_sb[:, cs])
                    if p >= NT // 2 - 2:
                        # Last two pairs: store halves on BOTH rings in
                        # parallel to shorten the end-of-kernel DMA drain.
                        nc.sync.dma_start(
                            outd[p, :, 0 : TILE // 2],
                            out_sb[:, p * TILE : p * TILE + TILE // 2],
                        )
                        nc.scalar.dma_start(
                            outd[p, :, TILE // 2 : TILE],
                            out_sb[:, p * TILE + TILE // 2 : (p + 1) * TILE],
                        )
                    elif p % 2 == 0:
                        nc.sync.dma_start(outd[p], out_sb[:, cs])
                    else:
                        nc.scalar.dma_start(outd[p], out_sb[:, cs])
                    x3_t[j - 1] = None
                    x3_t[j] = None

    nc.compile()
    return nc


_CACHED_NC = None
_CACHED_FLAGS = None


def kernel(h_v, h_w, e_vw, W1, b1, W2, b2, W3, b3, W4, b4):
    global LAST_RESULTS, _CACHED_NC, _CACHED_FLAGS

    h_w = np.asarray(h_w, np.float32)
    e_vw = np.asarray(e_vw, np.float32)
    W1 = np.asarray(W1, np.float32)
    W2 = np.asarray(W2, np.float32)
    W3 = np.asarray(W3, np.float32)
    W4 = np.asarray(W4, np.float32)
    b1 = np.asarray(b1, np.float32)
    b2 = np.asarray(b2, np.float32)
    b3 = np.asarray(b3, np.float32)
    b4 = np.asarray(b4, np.float32)

    # Host-side weight transform (exact reassociation of the reference math).
    W4s = W4.reshape(HID3, OUT_F, IN_F).sum(axis=2)
    b4s = b4.reshape(OUT_F, IN_F).sum(axis=1)
    s = h_w.reshape(-1)

    w3p = np.concatenate([W3[0:128], W3[128:256]], axis=1)  # [128, 256]
    bb = np.zeros((128, 5), np.float32)
    bb[:, 0] = b1
    bb[:, 1] = b2[0:128]
    bb[:, 2] = b2[128:256]
    bb[:, 3] = b3
    bb[0:OUT_F, 4] = b4s

    w4A = np.concatenate([W4s, np.zeros((HID3, 64), np.float32)], axis=1)
    w4B = np.concatenate([np.zeros((HID3, 64), np.float32), W4s], axis=1)
    b4blk = np.zeros((128, 128), np.float32)
    b4blk[0, 0:64] = b4s
    b4blk[1, 64:128] = b4s
    wpack = np.concatenate([W2, w3p, w4A, w4B, b4blk], axis=1)
    weights_map = {
        "w1d": np.ascontiguousarray(W1.astype(NP_BF16)),
        "wpd": np.ascontiguousarray(wpack, np.float32),
        "auxd": bb,
    }

    in_maps = []
    for c in range(N_CORES):
        sl = slice(c * E_LOC, (c + 1) * E_LOC)
        e_loc = e_vw[sl]                       # [4096, 32]
        s_loc = s[sl]                          # [4096]
        e_t = np.ascontiguousarray(e_loc.T.astype(NP_BF16))   # [32, 4096]
        # pair-stacked s broadcast: [128, 2048]
        s_pairs = s_loc.reshape(NT // 2, 2, TILE)           # [pair, half, 512]
        s_bcast = np.empty((128, E_LOC // 2), np.float32)
        s_bcast[0:64] = s_pairs[:, 0, :].reshape(-1)[None, :]
        s_bcast[64:128] = s_pairs[:, 1, :].reshape(-1)[None, :]
        sr2 = np.ascontiguousarray(
            np.stack([s_bcast[0], s_bcast[64]]), np.float32
        )  # [2, 2048]
        in_maps.append(
            {
                "e_t": e_t,
                "s_b": np.ascontiguousarray(s_bcast.astype(NP_BF16)),
                "srd": sr2,
                **weights_map,
            }
        )

    flags = (
        bool(np.array_equal(b2[0:128], b2[128:256])),
        bool(np.any(b4s != 0.0)),
    )
    if _CACHED_NC is None or _CACHED_FLAGS != flags:
        _CACHED_NC = _build_bass(
            b2_halves_equal=flags[0], b4_nonzero=flags[1]
        )
        _CACHED_FLAGS = flags
    nc = _CACHED_NC

    trace = bool(int(os.environ.get("KERNEL_TRACE", "0")))
    res = run_bass_kernel_spmd(
        nc, in_maps, core_ids=list(range(N_CORES)), trace=trace
    )
    LAST_RESULTS = res

    out = np.empty((E, OUT_F), np.float32)
    for c in range(N_CORES):
        o = res.results[c]["outd"]             # [4, 128, 512] bf16
        of = np.asarray(o, dtype=np.float32)
        base = c * E_LOC
        for p in range(NT // 2):
            out[base + 2 * p * TILE : base + (2 * p + 1) * TILE] = of[p, 0:64].T
            out[base + (2 * p + 1) * TILE : base + (2 * p + 2) * TILE] = (
                of[p, 64:128].T
            )
    return out
